# revision 2
# baseline (speedup 1.0000x reference)
"""AttnTopKPool Trainium2 kernel.

reference:
    w_mean = mean(w, axis=1)          # [B, S, S] -> [B, S]
    idx    = top_k(w_mean, 16)        # [B, 16]
    out    = x[b, :, idx[b]]          # [B, F, 16]

Strategy (8 NeuronCores, batch-parallel, 4 batches each):
  - host: transpose x to x_t[b, s, f] so the device gather is a contiguous
    row gather; slice w and x_t per core.
  - device: stream all 64 MiB of w per core on the sync HWDGE queue.
    Measured: a single queue with 4 MiB loads and deep buffering streams
    at ~425 GB/s/core under 8-way SPMD (multi-queue adds nothing; the
    per-core wall is aggregate). The kernel is memory-bound, so the whole
    game is (a) never letting the sync queue wait on a freshly-fired
    semaphore and (b) minimizing serial work after the last byte lands.
  - reduction at QUARTER granularity: each pair of row-group loads is
    summed by DVE in [128, 2048] quarter-adds (~2.3 us each) into small
    paq tiles, each immediately consumed by 4 fp32 [128,512] matmuls
    against a ones vector accumulating into the batch's [1, 2048] PSUM
    tile (bank = s-block). Fine granularity keeps PE fed early and
    shrinks SBUF so wt can hold bufs=3.
  - the DVE queue is in-order, so top-k ops (11.4 us/batch) are emitted
    INTERLEAVED with the next batch's quarter-adds: pass 1 after the
    next batch's first pair, pass 2 after its second pair. This removes
    the convoy where a top-k waiting on PE blocked the adds that free
    stream buffers (the baseline lost ~18 us of stream to that, plus a
    ~20 us PE backlog in the tail).
  - tail: the last batch's slots 14/15 are streamed as four 2 MiB
    half-loads pair-added as 2-quarter groups, so after the final byte
    only ~2 quarter-adds + 8 small matmuls precede top-k.
  - top-16 via DVE max8 / max_index on the [1, 2048] PSUM row;
    match_replace streams PSUM -> SBUF for ranks 9-16. First 8 gathers
    issue between the two passes.
  - gather: per index, reg_load into a register and issue a
    dynamic-offset DMA copying that 4 KiB row of x_t[b] straight
    DRAM->DRAM into the output, on scalar+gpsimd (never the streaming
    sync queue; +sync for the last batch once the stream is done).
  - out per core: [64, 1024] = (b_loc*16 + k, f); host reassembles to
    [B, F, K].
"""

import numpy as np

B, F, S, K = 32, 1024, 2048, 16
N_CORES = 8
B_LOC = B // N_CORES  # 4
P = 128
ROWS_PER_PART = 4          # w rows per SBUF partition in one big load
LOAD_FREE = ROWS_PER_PART * S   # 8192 floats = 32 KiB per partition
HALF_FREE = LOAD_FREE // 2      # 4096: 2 MiB half-loads for the tail
QF = S                     # 2048: quarter-add free size
MM_N = 512                 # one PSUM bank of fp32
NQ = QF // MM_N            # 4 psum column slices per quarter
NEG = -3.0e38              # below any column sum

_cached_nc = None

# test-only knobs (harness leaves these at defaults)
TRACE = False
_last_results = None


def _build_nc():
    from concourse import bacc, bass, mybir, tile

    f32 = mybir.dt.float32
    u32 = mybir.dt.uint32

    nc = bacc.Bacc("TRN2", target_bir_lowering=False, debug=False)

    w_d = nc.dram_tensor("w", [B_LOC, S, S], f32, kind="ExternalInput")
    xt_d = nc.dram_tensor("xt", [B_LOC, S, F], f32, kind="ExternalInput")
    out_d = nc.dram_tensor("out", [B_LOC * K, F], f32, kind="ExternalOutput")

    w_rows = w_d[:].rearrange("b r s -> (b r) s")
    # big view: [16, 128, 8192]; partition p of slot t holds rows (512t + 4p ..+3)
    w_big = w_rows.rearrange("(t p fr) s -> t p (fr s)", p=P, fr=ROWS_PER_PART)
    LAST = B_LOC - 1

    with tile.TileContext(nc) as tc:
        with (
            tc.tile_pool(name="wtp", bufs=3) as wtp,
            tc.tile_pool(name="whp", bufs=4) as whp,
            tc.tile_pool(name="paq", bufs=4) as paqp,
            tc.tile_pool(name="smpool", bufs=1) as smpool,
            tc.tile_pool(name="pspool", bufs=2, space="PSUM") as pspool,
            tc.tile_pool(name="tk", bufs=1) as tk,
        ):
            ones = tk.tile([P, 1], f32)
            nc.vector.memset(ones[:], 1.0)

            ps_of = {}

            def emit_quarter(b, qidx, wa, wb, off, nquarters=4):
                """One quarter: DVE add of [128, QF] slices of wa/wb into a
                paq tile, then 4 matmul chunks into psum banks 0..3."""
                ps = ps_of[b]
                pa = paqp.tile([P, QF], f32, name=f"pa{b}_{qidx}", tag="paq")
                lo, hi = off, off + QF
                nc.vector.tensor_add(pa[:], wa[:, lo:hi], wb[:, lo:hi])
                for c in range(NQ):
                    nc.tensor.matmul(
                        ps[:, c * MM_N : (c + 1) * MM_N],
                        ones[:],
                        pa[:, c * MM_N : (c + 1) * MM_N],
                        start=(qidx == 0),
                        stop=(qidx == 7),
                    )

            def gather(b, k, etype, eng, gidx):
                regs = nc.alloc_registers(name=f"ri{b}_{k}", engines=(etype,))
                reg = list(regs)[0]
                eng.reg_load(reg, gidx[0:1, k : k + 1])
                val = eng.snap(reg, donate=True, min_val=0, max_val=S - 1)
                eng.dma_start(
                    out_d[b * K + k : b * K + k + 1, :],
                    xt_d[b][bass.ds(val, 1), :],
                )

            def make_topk(b):
                """Returns (pass1, pass2) closures for batch b's top-16."""
                ps = ps_of[b]
                gidx = tk.tile([1, K], u32, name=f"gidx{b}")
                m8a = tk.tile([1, 8], f32, name=f"m8a{b}")
                m8b = tk.tile([1, 8], f32, name=f"m8b{b}")
                engs = [
                    (mybir.EngineType.Activation, nc.scalar),
                    (mybir.EngineType.Pool, nc.gpsimd),
                ]
                if b == LAST:
                    engs.append((mybir.EngineType.SP, nc.sync))

                def pass1():
                    nc.vector.max(m8a[:], ps[:])
                    nc.vector.max_index(gidx[:, 0:8], m8a[:], ps[:])
                    for k in range(8):
                        gather(b, k, *engs[k % len(engs)], gidx)

                def pass2():
                    sums = smpool.tile([1, S], f32, name=f"sums{b}", tag="sums")
                    nc.vector.match_replace(sums[:], m8a[:], ps[:], NEG)
                    nc.vector.max(m8b[:], sums[:])
                    nc.vector.max_index(gidx[:, 8:16], m8b[:], sums[:])
                    for k in range(8, K):
                        gather(b, k, *engs[k % len(engs)], gidx)

                return pass1, pass2

            prev = None  # pending (pass1, pass2) of the previous batch
            for b in range(B_LOC):
                ps_of[b] = pspool.tile([1, S], f32, name=f"ps{b}", tag="ps")

                # --- first pair: two 4 MiB loads, 4 quarter-adds ---
                wa = wtp.tile([P, LOAD_FREE], f32, name=f"wa{b}", tag="wt")
                nc.sync.dma_start(wa[:], w_big[4 * b])
                wb = wtp.tile([P, LOAD_FREE], f32, name=f"wb{b}", tag="wt")
                nc.sync.dma_start(wb[:], w_big[4 * b + 1])
                for q in range(4):
                    emit_quarter(b, q, wa, wb, q * QF)

                if prev is not None:
                    prev[0]()  # previous batch's top-k pass 1 + gathers

                # --- second pair ---
                if b != LAST:
                    wc = wtp.tile([P, LOAD_FREE], f32, name=f"wc{b}", tag="wt")
                    nc.sync.dma_start(wc[:], w_big[4 * b + 2])
                    wd = wtp.tile([P, LOAD_FREE], f32, name=f"wd{b}", tag="wt")
                    nc.sync.dma_start(wd[:], w_big[4 * b + 3])
                    for q in range(4):
                        emit_quarter(b, 4 + q, wc, wd, q * QF)
                else:
                    # tail: slots 14/15 streamed as 2 MiB half-loads, each
                    # half-pair adding into 2 quarters
                    for j, slot in enumerate((4 * b + 2, 4 * b + 3)):
                        ha = whp.tile([P, HALF_FREE], f32, name=f"ha{j}", tag="wh")
                        nc.sync.dma_start(ha[:], w_big[slot][:, 0:HALF_FREE])
                        hb = whp.tile([P, HALF_FREE], f32, name=f"hb{j}", tag="wh")
                        nc.sync.dma_start(hb[:], w_big[slot][:, HALF_FREE:])
                        for q in range(2):
                            emit_quarter(b, 4 + 2 * j + q, ha, hb, q * QF)

                if prev is not None:
                    prev[1]()  # previous batch's top-k pass 2 + gathers

                prev = make_topk(b)

            prev[0]()
            prev[1]()

    nc.compile()
    return nc


def _get_nc():
    global _cached_nc
    if _cached_nc is None:
        _cached_nc = _build_nc()
    return _cached_nc


def kernel(x: np.ndarray, w: np.ndarray) -> np.ndarray:
    from concourse import bass_utils

    x = np.asarray(x, dtype=np.float32)
    w = np.asarray(w, dtype=np.float32)
    x_t = np.ascontiguousarray(x.transpose(0, 2, 1))  # [B, S, F]

    nc = _get_nc()
    in_maps = [
        {
            "w": np.ascontiguousarray(w[c * B_LOC : (c + 1) * B_LOC]),
            "xt": x_t[c * B_LOC : (c + 1) * B_LOC],
        }
        for c in range(N_CORES)
    ]
    res = bass_utils.run_bass_kernel_spmd(
        nc, in_maps, list(range(N_CORES)), trace=TRACE
    )
    global _last_results
    _last_results = res
    out = np.concatenate([res.results[c]["out"] for c in range(N_CORES)], axis=0)
    # [B*K, F] -> [B, K, F] -> [B, F, K]
    return np.ascontiguousarray(out.reshape(B, K, F).transpose(0, 2, 1))


# revision 3
# speedup vs baseline: 1.0378x; 1.0378x over previous
"""AttnTopKPool Trainium2 kernel.

reference:
    w_mean = mean(w, axis=1)          # [B, S, S] -> [B, S]
    idx    = top_k(w_mean, 16)        # [B, 16]
    out    = x[b, :, idx[b]]          # [B, F, 16]

Strategy (8 NeuronCores, batch-parallel, 4 batches each):
  - host: transpose x to x_t[b, s, f] so the device gather is a contiguous
    row gather; slice w and x_t per core.
  - device: stream all 64 MiB of w per core on the sync HWDGE queue as
    sixteen 4 MiB [128, 8192] loads. HW-measured: one queue with 4 MiB
    loads streams at ~425 GB/s/core under 8-way SPMD, but ONLY if the
    queue keeps >=2 loads of depth: a load whose buffer is freed less
    than ~2 us before its natural start never queues behind the
    in-flight transfer and the stream degrades to ~330 GB/s
    (issue-latency per load). wt bufs=4 makes every buffer free ~9 us
    before its reuse; the sync queue then never sees a fresh semaphore.
  - reduction at QUARTER granularity: each pair of loads is summed by
    DVE in [128, 2048] quarter-adds (~2.3 us) into paq tiles, each
    immediately consumed by 4 fp32 [128,512] matmuls against a ones
    vector accumulating into the batch's [1, 2048] PSUM row (bank =
    s-block; 8 accumulations per bank per batch). fp32 is required: the
    smallest top-16 gap on these U(0,1) column sums is ~4e-3 while
    fp32r/TF32 matmul error is ~5e-3 (HW-probed).
  - the DVE queue is in-order, so top-k ops (11.4 us/batch) are emitted
    INTERLEAVED with the next batch's quarter-adds (pass 1 after its
    first pair, match_replace + pass 2 around its second pair). This
    removes the convoy where a top-k waiting on PE blocked the adds
    that free stream buffers.
  - tail: the final slot's 4 MiB is loaded as four 1 MiB sub-loads into
    disjoint quarters of one wt tile, so the last pair's quarter-adds
    chase sub-arrivals and only ~one add + 4 small matmuls + top-k
    remain after the final byte.
  - top-16 via DVE max8 / max_index straight out of PSUM; match_replace
    streams PSUM -> SBUF for ranks 9-16. First 8 gathers issue between
    the two passes.
  - gather: per index, reg_load into a register and issue a
    dynamic-offset DMA copying that 4 KiB row of x_t[b] straight
    DRAM->DRAM into the output, on scalar+gpsimd (never the streaming
    sync queue; +sync for the last batch once the stream is done).
  - out per core: [64, 1024] = (b_loc*16 + k, f); host reassembles to
    [B, F, K].
"""

import numpy as np

B, F, S, K = 32, 1024, 2048, 16
N_CORES = 8
B_LOC = B // N_CORES  # 4
P = 128
ROWS_PER_PART = 4          # w rows per SBUF partition in one big load
LOAD_FREE = ROWS_PER_PART * S   # 8192 floats = 32 KiB per partition
QF = S                     # 2048: quarter-add free size
MM_N = 512                 # one PSUM bank of fp32
NQ = QF // MM_N            # 4 psum column slices per quarter
NEG = -3.0e38              # below any column sum

_cached_nc = None

# test-only knobs (harness leaves these at defaults)
TRACE = False
_last_results = None


def _build_nc():
    from concourse import bacc, bass, mybir, tile

    f32 = mybir.dt.float32
    u32 = mybir.dt.uint32

    nc = bacc.Bacc("TRN2", target_bir_lowering=False, debug=False)

    w_d = nc.dram_tensor("w", [B_LOC, S, S], f32, kind="ExternalInput")
    xt_d = nc.dram_tensor("xt", [B_LOC, S, F], f32, kind="ExternalInput")
    out_d = nc.dram_tensor("out", [B_LOC * K, F], f32, kind="ExternalOutput")

    w_rows = w_d[:].rearrange("b r s -> (b r) s")
    # big view: [16, 128, 8192]; partition p of slot t holds rows (512t + 4p ..+3)
    w_big = w_rows.rearrange("(t p fr) s -> t p (fr s)", p=P, fr=ROWS_PER_PART)
    LAST = B_LOC - 1

    with tile.TileContext(nc) as tc:
        with (
            tc.tile_pool(name="wtp", bufs=4) as wtp,
            tc.tile_pool(name="paq", bufs=6) as paqp,
            tc.tile_pool(name="smpool", bufs=1) as smpool,
            tc.tile_pool(name="pspool", bufs=2, space="PSUM") as pspool,
            tc.tile_pool(name="tk", bufs=1) as tk,
        ):
            ones = tk.tile([P, 1], f32)
            nc.vector.memset(ones[:], 1.0)

            ps_of = {}

            def emit_quarter(b, qidx, wa, wb, off):
                """One quarter: DVE add of [128, QF] slices of wa/wb into a
                paq tile, then 4 matmul chunks into psum banks 0..3."""
                ps = ps_of[b]
                pa = paqp.tile([P, QF], f32, name=f"pa{b}_{qidx}", tag="paq")
                lo, hi = off, off + QF
                nc.vector.tensor_add(pa[:], wa[:, lo:hi], wb[:, lo:hi])
                for c in range(NQ):
                    nc.tensor.matmul(
                        ps[:, c * MM_N : (c + 1) * MM_N],
                        ones[:],
                        pa[:, c * MM_N : (c + 1) * MM_N],
                        start=(qidx == 0),
                        stop=(qidx == 7),
                    )

            def gather(b, k, etype, eng, gidx):
                regs = nc.alloc_registers(name=f"ri{b}_{k}", engines=(etype,))
                reg = list(regs)[0]
                eng.reg_load(reg, gidx[0:1, k : k + 1])
                val = eng.snap(reg, donate=True, min_val=0, max_val=S - 1)
                eng.dma_start(
                    out_d[b * K + k : b * K + k + 1, :],
                    xt_d[b][bass.ds(val, 1), :],
                )

            def make_topk(b):
                """Returns (pass1, mr, pass2) closures for batch b's top-16."""
                ps = ps_of[b]
                gidx = tk.tile([1, K], u32, name=f"gidx{b}")
                m8a = tk.tile([1, 8], f32, name=f"m8a{b}")
                m8b = tk.tile([1, 8], f32, name=f"m8b{b}")
                sums = smpool.tile([1, S], f32, name=f"sums{b}", tag="sums")
                engs = [
                    (mybir.EngineType.Activation, nc.scalar),
                    (mybir.EngineType.Pool, nc.gpsimd),
                ]
                if b == LAST:
                    engs.append((mybir.EngineType.SP, nc.sync))

                def pass1():
                    nc.vector.max(m8a[:], ps[:])
                    nc.vector.max_index(gidx[:, 0:8], m8a[:], ps[:])
                    for k in range(8):
                        gather(b, k, *engs[k % len(engs)], gidx)

                def mr():
                    nc.vector.match_replace(sums[:], m8a[:], ps[:], NEG)

                def pass2():
                    nc.vector.max(m8b[:], sums[:])
                    nc.vector.max_index(gidx[:, 8:16], m8b[:], sums[:])
                    for k in range(8, K):
                        gather(b, k, *engs[k % len(engs)], gidx)

                return pass1, mr, pass2

            prev = None  # pending (pass1, mr, pass2) of the previous batch
            for b in range(B_LOC):
                ps_of[b] = pspool.tile([1, S], f32, name=f"ps{b}", tag="ps")

                # --- first pair: two 4 MiB loads, 4 quarter-adds ---
                wa = wtp.tile([P, LOAD_FREE], f32, name=f"wa{b}", tag="wt")
                nc.sync.dma_start(wa[:], w_big[4 * b])
                wb = wtp.tile([P, LOAD_FREE], f32, name=f"wb{b}", tag="wt")
                nc.sync.dma_start(wb[:], w_big[4 * b + 1])
                for q in range(4):
                    emit_quarter(b, q, wa, wb, q * QF)

                if prev is not None:
                    prev[0]()  # previous batch's top-k pass 1 + gathers

                # --- second pair ---
                wc = wtp.tile([P, LOAD_FREE], f32, name=f"wc{b}", tag="wt")
                nc.sync.dma_start(wc[:], w_big[4 * b + 2])
                wd = wtp.tile([P, LOAD_FREE], f32, name=f"wd{b}", tag="wt")
                if b != LAST:
                    nc.sync.dma_start(wd[:], w_big[4 * b + 3])
                    for q in range(4):
                        emit_quarter(b, 4 + q, wc, wd, q * QF)
                    if prev is not None:
                        prev[1]()
                        prev[2]()
                else:
                    # final slot streamed as four 1 MiB sub-loads into
                    # disjoint quarters of wd, so each quarter-add runs as
                    # soon as its sub-load lands; the previous batch's
                    # match_replace slots between adds to keep DVE clear
                    # for this batch's top-k right after the last matmul.
                    for q in range(4):
                        nc.sync.dma_start(
                            wd[:, q * QF : (q + 1) * QF],
                            w_big[4 * b + 3][:, q * QF : (q + 1) * QF],
                        )
                    for q in range(2):
                        emit_quarter(b, 4 + q, wc, wd, q * QF)
                    if prev is not None:
                        prev[1]()
                    for q in range(2, 4):
                        emit_quarter(b, 4 + q, wc, wd, q * QF)
                    if prev is not None:
                        prev[2]()

                prev = make_topk(b)

            prev[0]()
            prev[1]()
            prev[2]()

    nc.compile()
    return nc


def _get_nc():
    global _cached_nc
    if _cached_nc is None:
        _cached_nc = _build_nc()
    return _cached_nc


def kernel(x: np.ndarray, w: np.ndarray) -> np.ndarray:
    from concourse import bass_utils

    x = np.asarray(x, dtype=np.float32)
    w = np.asarray(w, dtype=np.float32)
    x_t = np.ascontiguousarray(x.transpose(0, 2, 1))  # [B, S, F]

    nc = _get_nc()
    in_maps = [
        {
            "w": np.ascontiguousarray(w[c * B_LOC : (c + 1) * B_LOC]),
            "xt": x_t[c * B_LOC : (c + 1) * B_LOC],
        }
        for c in range(N_CORES)
    ]
    res = bass_utils.run_bass_kernel_spmd(
        nc, in_maps, list(range(N_CORES)), trace=TRACE
    )
    global _last_results
    _last_results = res
    out = np.concatenate([res.results[c]["out"] for c in range(N_CORES)], axis=0)
    # [B*K, F] -> [B, K, F] -> [B, F, K]
    return np.ascontiguousarray(out.reshape(B, K, F).transpose(0, 2, 1))
